# revision 33
# baseline (speedup 1.0000x reference)
"""Discounted cumsum along S for tensor (8, 16, 4096, 64), gamma (16,).

y[b,h,t,d] = gamma[h] * y[b,h,t-1,d] + x[b,h,t,d],  y[...,-1,:] = 0

Strategy (8 NeuronCores, shard over B):
  - core b handles batch b: slab (16, 4096, 64) f32, 16 MiB in / 16 MiB out.
  - Per core: S=4096 split into T tiles; within a tile, partitions are
    (h, blk) = 16 heads x 8 sequence-blocks, each partition holding W
    consecutive s-steps x 64 d contiguous in DRAM (fast DMA).
  - Two-pass hierarchical scan:
      pass 1: per-d `tensor_tensor_scan` (DVE) -> per-block local scans;
              keep only each block's last element (the block carry c).
      carry:  block-diagonal (per-h) triangular fp32 matmuls on TensorE
              propagate carries across blocks/tiles: C = sum TRI_ut^T @ c_u.
      pass 2: re-scan with initial = C[:, d] (from PSUM), writing the
              final y in-place over the x tile, then DMA out.
  - gamma-power matrices are precomputed on the host (gamma-derived
    constants only; all x-dependent work happens on device).
"""

import numpy as np

import concourse.bacc as bacc
import concourse.bass as bass
import concourse.mybir as mybir
import concourse.tile as tile
from concourse.bass_utils import run_bass_kernel_spmd

F32 = mybir.dt.float32

B, H, S, D = 8, 16, 4096, 64
N_CORES = 8

# Per-core tiling: T s-tiles, BLK sequence blocks per tile, W steps per block.
T_TILES = 4
BLK = 8
W = S // (T_TILES * BLK)  # 128
NPART = H * BLK  # 128
FREE = BLK and W * D  # free elems per partition per tile (W*D)


def _pair_index(u, t):
    # index of (u, t), u <= t, in the stacked TRI tensor
    return t * (t + 1) // 2 + u


def build_program(T=T_TILES, blk=BLK, w=None, h=H, d=D):
    """Build the SPMD Bass program (same on every core)."""
    if w is None:
        w = S // (T * blk)
    s = T * blk * w
    npart = h * blk
    free = w * d
    npairs = T * (T + 1) // 2

    nc = bacc.Bacc("TRN2", target_bir_lowering=False, enable_partition_id=False)

    x_ext = nc.declare_dram_parameter("x", [h, s, d], F32, isOutput=False)
    gam_ext = nc.declare_dram_parameter("gamma_tile", [npart, w], F32, isOutput=False)
    tri_ext = nc.declare_dram_parameter(
        "tri", [npairs, npart, npart], F32, isOutput=False
    )
    y_ext = nc.declare_dram_parameter("y", [h, s, d], F32, isOutput=True)

    # DRAM views: tile t -> (h, blk, w*d); iteration order (h, blk, wd)
    # matches the SBUF tile's (partition p = h*blk + blk, free = w*d) order.
    x_v = x_ext[:].rearrange(
        "h (t blk w) d -> t h blk (w d)", t=T, blk=blk, w=w
    )
    y_v = y_ext[:].rearrange(
        "h (t blk w) d -> t h blk (w d)", t=T, blk=blk, w=w
    )
    tri_v = tri_ext[:].rearrange("n k m -> k n m")
    dump_ext = nc.dram_tensor("obs_dump", [npart, 1], F32)
    dump_v = dump_ext[:]

    with tile.TileContext(nc) as tc:
        with (
            tc.tile_pool(name="xp", bufs=2) as xp,
            tc.tile_pool(name="yp", bufs=2) as yp,
            tc.tile_pool(name="scratch", bufs=1) as sp,
            tc.tile_pool(name="consts", bufs=1) as cp,
            tc.tile_pool(name="guards", bufs=4) as gp,
            tc.tile_pool(name="psum", bufs=2, space="PSUM") as pp,
        ):
            gam = cp.tile([npart, w], F32)
            nc.sync.dma_start(gam[:], gam_ext[:])
            tri_sb = cp.tile([npart, npairs * npart], F32)
            nc.sync.dma_start(tri_sb[:], tri_v)
            # all block-carry vectors, one (npart, d) column-block per tile
            c_all = cp.tile([npart, T * d], F32)
            # SBUF copy of the propagated carries (scan `initial` reads this;
            # the copy also absorbs the PE->DVE semaphore wait, which the
            # scan ISA op cannot carry)
            cprop = cp.tile([npart, T * d], F32)

            # Engine instructions can encode only ONE sync wait. Guard
            # instructions absorb each DMA-completion wait so the scan /
            # copy / matmul instructions that follow need at most one
            # (engine-sem) wait each.
            g0 = gp.tile([npart, 1], F32, tag="guard")
            nc.vector.tensor_copy(g0[:], gam[:, 0:1])
            pe_guard = pp.tile([1, 1], F32)
            nc.tensor.matmul(
                pe_guard[:], tri_sb[:, 0:1], tri_sb[:, 0:1], start=True, stop=True
            )

            mult = mybir.AluOpType.mult
            add = mybir.AluOpType.add

            scratch = sp.tile([npart, free], F32)
            # scratch viewed (p, d, w): pass-1 scan d writes a contiguous
            # w-run; the block carries are then the (p, d, w=-1) slice.
            scr3 = scratch[:].rearrange("p (dd ww) -> p dd ww", dd=d)

            for t in range(T):
                xt = xp.tile([npart, free], F32)
                nc.sync.dma_start(xt[:], x_v[t])
                xt3 = xt[:].rearrange("p (ww dd) -> p ww dd", dd=d)
                # guard: absorb the in-DMA wait on a copy, not a scan
                gt = gp.tile([npart, 1], F32, tag="guard")
                nc.vector.tensor_copy(gt[:], xt[:, 0:1])

                # pass 1: local scans (initial=0), keep only block carries
                for dd in range(d):
                    nc.vector.tensor_tensor_scan(
                        out=scr3[:, dd, :],
                        data0=gam[:],
                        data1=xt3[:, :, dd],
                        initial=0.0,
                        op0=mult,
                        op1=add,
                    )
                nc.vector.tensor_copy(
                    c_all[:, t * d : (t + 1) * d], scr3[:, :, w - 1]
                )

                # carry propagation across blocks (and earlier tiles)
                C_t = pp.tile([npart, d], F32)
                for u in range(t + 1):
                    i = _pair_index(u, t)
                    nc.tensor.matmul(
                        C_t[:],
                        tri_sb[:, i * npart : (i + 1) * npart],
                        c_all[:, u * d : (u + 1) * d],
                        start=(u == 0),
                        stop=(u == t),
                    )

                # PSUM -> SBUF; also absorbs the PE->DVE wait
                nc.vector.tensor_copy(cprop[:, t * d : (t + 1) * d], C_t[:])

                # pass 2: true scan with per-block initial carries, into a
                # dedicated y tile (out-DMA then depends only on DVE).
                # Wait-budget choreography (one sync wait per engine instr):
                #  - guard copy reading cprop absorbs the PE-chain DVE wait,
                #  - a scalar_tensor_tensor computes y[w=0] for d=0 into yt,
                #    absorbing the out-DMA slot-reuse WAR wait,
                #  - the d=0 scan then covers w in [1, W) with initial=y[0].
                yt = yp.tile([npart, free], F32)
                yt3 = yt[:].rearrange("p (ww dd) -> p ww dd", dd=d)
                ga = gp.tile([npart, 1], F32, tag="guard")
                nc.vector.tensor_copy(ga[:], cprop[:, t * d : t * d + 1])
                nc.vector.scalar_tensor_tensor(
                    out=yt[:, 0:1],
                    in0=cprop[:, t * d : t * d + 1],
                    scalar=gam[:, 0:1],
                    in1=xt[:, 0:1],
                    op0=mult,
                    op1=add,
                )
                for dd in range(d):
                    if dd == 0:
                        nc.vector.tensor_tensor_scan(
                            out=yt3[:, 1:, dd],
                            data0=gam[:, 1:],
                            data1=xt3[:, 1:, dd],
                            initial=yt[:, 0:1],
                            op0=mult,
                            op1=add,
                        )
                    else:
                        nc.vector.tensor_tensor_scan(
                            out=yt3[:, :, dd],
                            data0=gam[:],
                            data1=xt3[:, :, dd],
                            initial=cprop[:, t * d + dd : t * d + dd + 1],
                            op0=mult,
                            op1=add,
                        )

                # observer DMA: reads yt's last-written column so it carries
                # the DVE-complete wait; the real out-DMA right after then
                # needs only (at most) one not-yet-observed DMA-lane wait.
                nc.sync.dma_start(dump_v, yt[:, free - 1 : free])
                nc.sync.dma_start(y_v[t], yt[:])

    # Run Bacc's lowering pipeline (incl. generate_event_semaphores, which
    # splits multi-sem waits to satisfy the one-wait-per-instruction
    # hardware constraint); the axon/pjrt exec path does not finalize
    # prebuilt modules itself.
    nc.finalize()
    return nc


def host_aux(gamma, T=T_TILES, blk=BLK, w=None):
    """gamma-derived constant inputs (host precompute)."""
    if w is None:
        w = S // (T * blk)
    h = gamma.shape[0]
    npart = h * blk
    g64 = gamma.astype(np.float64)

    gamma_tile = np.repeat(gamma.astype(np.float32), blk)[:, None] * np.ones(
        (1, w), np.float32
    )

    nblk = T * blk
    npairs = T * (T + 1) // 2
    tri = np.zeros((npairs, npart, npart), np.float64)
    # global block index G = t*blk + p; carry into block G from block G':
    # decay gamma^(w * (G - 1 - G')) for G' <= G - 1 (per head, block-diag)
    gw = g64**w  # per-head decay across one block
    with np.errstate(under="ignore"):
        for t in range(T):
            for u in range(t + 1):
                m = tri[_pair_index(u, t)]
                for hh in range(h):
                    for q in range(blk):
                        for p in range(blk):
                            gq = u * blk + q
                            gp = t * blk + p
                            if gq <= gp - 1:
                                m[hh * blk + q, hh * blk + p] = gw[hh] ** (
                                    gp - 1 - gq
                                )
    return gamma_tile.astype(np.float32), tri.astype(np.float32)


_CACHE = {}


def kernel(tensor, gamma):
    tensor = np.asarray(tensor, dtype=np.float32)
    gamma = np.asarray(gamma, dtype=np.float32)
    assert tensor.shape == (B, H, S, D), tensor.shape

    if "nc" not in _CACHE:
        _CACHE["nc"] = build_program()
    nc = _CACHE["nc"]

    gamma_tile, tri = host_aux(gamma)
    in_maps = [
        {"x": np.ascontiguousarray(tensor[b]), "gamma_tile": gamma_tile, "tri": tri}
        for b in range(N_CORES)
    ]
    res = run_bass_kernel_spmd(nc, in_maps, list(range(N_CORES)))
    out = np.stack([np.asarray(res.results[b]["y"]) for b in range(N_CORES)], axis=0)
    return out


# revision 34
# speedup vs baseline: 1.1851x; 1.1851x over previous
"""Discounted cumsum along S for tensor (8, 16, 4096, 64), gamma (16,).

y[b,h,t,d] = gamma[h] * y[b,h,t-1,d] + x[b,h,t,d],  y[...,-1,:] = 0

Strategy (8 NeuronCores, shard over B):
  - core b handles batch b: slab (16, 4096, 64) f32, 16 MiB in / 16 MiB out.
  - Per core: S=4096 split into T tiles; within a tile, partitions are
    (h, blk) = 16 heads x 8 sequence-blocks, each partition holding W
    consecutive s-steps x 64 d contiguous in DRAM (fast DMA).
  - Two-pass hierarchical scan:
      pass 1: per-d `tensor_tensor_scan` (DVE) -> per-block local scans
              written into the y tile (used as scratch); keep only each
              block's last element (the block carry c).
      carry:  block-diagonal (per-h) triangular fp32 matmuls on TensorE
              propagate carries across blocks/tiles: C = sum TRI_ut^T @ c_u.
      pass 2: re-scan with initial = C[:, d], overwriting the y tile,
              then DMA out.
  - in-DMAs ride the Sync DGE, out-DMAs the Scalar DGE: separate FIFOs,
    so prefetches are not head-of-line blocked by output drains.
  - gamma-power matrices are precomputed on the host (gamma-derived
    constants only; all x-dependent work happens on device).
"""

import os

import numpy as np

import concourse.bacc as bacc
import concourse.bass as bass
import concourse.mybir as mybir
import concourse.tile as tile
from concourse.bass_utils import run_bass_kernel_spmd

F32 = mybir.dt.float32

B, H, S, D = 8, 16, 4096, 64
N_CORES = 8

# Per-core tiling: T s-tiles, BLK sequence blocks per tile, W steps per block.
T_TILES = 4
BLK = 8
W = S // (T_TILES * BLK)  # 128
NPART = H * BLK  # 128


def _pair_index(u, t):
    # index of (u, t), u <= t, in the stacked TRI tensor
    return t * (t + 1) // 2 + u


def build_program(T=T_TILES, blk=BLK, w=None, h=H, d=D, gp_split=0):
    """Build the SPMD Bass program (same on every core).

    gp_split: number of d-chains (per pass, per tile) offloaded to GPSIMD.
    """
    if w is None:
        w = S // (T * blk)
    s = T * blk * w
    npart = h * blk
    free = w * d
    npairs = T * (T + 1) // 2

    nc = bacc.Bacc("TRN2", target_bir_lowering=False, enable_partition_id=False)

    x_ext = nc.declare_dram_parameter("x", [h, s, d], F32, isOutput=False)
    gam_ext = nc.declare_dram_parameter("gamma_tile", [npart, w], F32, isOutput=False)
    tri_ext = nc.declare_dram_parameter(
        "tri", [npairs, npart, npart], F32, isOutput=False
    )
    y_ext = nc.declare_dram_parameter("y", [h, s, d], F32, isOutput=True)

    # DRAM views: tile t -> (h, blk, w*d); iteration order (h, blk, wd)
    # matches the SBUF tile's (partition p = h*blk + blk, free = w*d) order.
    x_v = x_ext[:].rearrange("h (t blk w) d -> t h blk (w d)", t=T, blk=blk, w=w)
    y_v = y_ext[:].rearrange("h (t blk w) d -> t h blk (w d)", t=T, blk=blk, w=w)
    tri_v = tri_ext[:].rearrange("n k m -> k n m")

    mult = mybir.AluOpType.mult
    add = mybir.AluOpType.add

    with tile.TileContext(nc) as tc:
        with (
            tc.tile_pool(name="xp", bufs=3) as xp,
            tc.tile_pool(name="yp", bufs=2) as yp,
            tc.tile_pool(name="consts", bufs=1) as cp,
            tc.tile_pool(name="psum", bufs=2, space="PSUM") as pp,
        ):
            gam = cp.tile([npart, w], F32)
            nc.sync.dma_start(gam[:], gam_ext[:])
            tri_sb = cp.tile([npart, npairs * npart], F32)
            nc.sync.dma_start(tri_sb[:], tri_v)
            # all block-carry vectors, one (npart, d) column-block per tile
            c_all = cp.tile([npart, T * d], F32)
            # SBUF copy of the propagated carries (scan initial source)
            cprop = cp.tile([npart, T * d], F32)

            for t in range(T):
                xt = xp.tile([npart, free], F32)
                nc.sync.dma_start(xt[:], x_v[t])
                xt3 = xt[:].rearrange("p (ww dd) -> p ww dd", dd=d)

                # y tile doubles as pass-1 scratch: (p, d, w) view, each
                # pass-1 scan writes a contiguous w-run; block carries are
                # the (p, d, w=-1) slice. Pass 2 later overwrites it in
                # (p, w, d) order.
                yt = yp.tile([npart, free], F32)
                yt3 = yt[:].rearrange("p (ww dd) -> p ww dd", dd=d)
                scr3 = yt[:].rearrange("p (dd ww) -> p dd ww", dd=d)

                # pass 1: local scans (initial=0), keep only block carries
                for dd in range(d):
                    eng = nc.gpsimd if dd >= d - gp_split else nc.vector
                    eng.tensor_tensor_scan(
                        out=scr3[:, dd, :],
                        data0=gam[:],
                        data1=xt3[:, :, dd],
                        initial=0.0,
                        op0=mult,
                        op1=add,
                    )
                nc.vector.tensor_copy(
                    c_all[:, t * d : (t + 1) * d], scr3[:, :, w - 1]
                )

                # carry propagation across blocks (and earlier tiles)
                C_t = pp.tile([npart, d], F32)
                for u in range(t + 1):
                    i = _pair_index(u, t)
                    nc.tensor.matmul(
                        C_t[:],
                        tri_sb[:, i * npart : (i + 1) * npart],
                        c_all[:, u * d : (u + 1) * d],
                        start=(u == 0),
                        stop=(u == t),
                    )
                # PSUM -> SBUF (GPSIMD cannot read PSUM; scans read this)
                nc.vector.tensor_copy(cprop[:, t * d : (t + 1) * d], C_t[:])

                # pass 2: true scan with per-block initial carries
                for dd in range(d):
                    eng = nc.gpsimd if dd >= d - gp_split else nc.vector
                    eng.tensor_tensor_scan(
                        out=yt3[:, :, dd],
                        data0=gam[:],
                        data1=xt3[:, :, dd],
                        initial=cprop[:, t * d + dd : t * d + dd + 1],
                        op0=mult,
                        op1=add,
                    )

                # out-DMA on the Scalar DGE (independent FIFO from inputs)
                nc.scalar.dma_start(y_v[t], yt[:])

    # Run Bacc's lowering pipeline (incl. generate_event_semaphores, which
    # splits multi-sem waits to satisfy the one-wait-per-instruction
    # hardware constraint); the axon/pjrt exec path does not finalize
    # prebuilt modules itself.
    nc.finalize()
    return nc


def host_aux(gamma, T=T_TILES, blk=BLK, w=None):
    """gamma-derived constant inputs (host precompute)."""
    if w is None:
        w = S // (T * blk)
    h = gamma.shape[0]
    npart = h * blk
    g64 = gamma.astype(np.float64)

    gamma_tile = np.repeat(gamma.astype(np.float32), blk)[:, None] * np.ones(
        (1, w), np.float32
    )

    npairs = T * (T + 1) // 2
    tri = np.zeros((npairs, npart, npart), np.float64)
    # global block index G = t*blk + p; carry into block G from block G':
    # decay gamma^(w * (G - 1 - G')) for G' <= G - 1 (per head, block-diag)
    gw = g64**w  # per-head decay across one block
    with np.errstate(under="ignore"):
        for t in range(T):
            for u in range(t + 1):
                m = tri[_pair_index(u, t)]
                for hh in range(h):
                    for q in range(blk):
                        for p in range(blk):
                            gq = u * blk + q
                            gp = t * blk + p
                            if gq <= gp - 1:
                                m[hh * blk + q, hh * blk + p] = gw[hh] ** (
                                    gp - 1 - gq
                                )
    return gamma_tile.astype(np.float32), tri.astype(np.float32)


_CACHE = {}


def kernel(tensor, gamma):
    tensor = np.asarray(tensor, dtype=np.float32)
    gamma = np.asarray(gamma, dtype=np.float32)
    assert tensor.shape == (B, H, S, D), tensor.shape

    if "nc" not in _CACHE:
        _CACHE["nc"] = build_program(
            gp_split=int(os.environ.get("KERNEL_GP_SPLIT", "0"))
        )
    nc = _CACHE["nc"]

    gamma_tile, tri = host_aux(gamma)
    in_maps = [
        {"x": np.ascontiguousarray(tensor[b]), "gamma_tile": gamma_tile, "tri": tri}
        for b in range(N_CORES)
    ]
    res = run_bass_kernel_spmd(nc, in_maps, list(range(N_CORES)))
    out = np.stack([np.asarray(res.results[b]["y"]) for b in range(N_CORES)], axis=0)
    return out


# revision 38
# speedup vs baseline: 1.2166x; 1.0266x over previous
"""Discounted cumsum along S for tensor (8, 16, 4096, 64), gamma (16,).

y[b,h,t,d] = gamma[h] * y[b,h,t-1,d] + x[b,h,t,d],  y[...,-1,:] = 0

Strategy (8 NeuronCores, shard over B):
  - core b handles batch b: slab (16, 4096, 64) f32, 16 MiB in / 16 MiB out.
  - Per core: S=4096 split into T tiles; within a tile, partitions are
    (h, blk) = 16 heads x 8 sequence-blocks, each partition holding W
    consecutive s-steps x 64 d contiguous in DRAM (fast DMA).
  - Two-pass hierarchical scan:
      pass 1: per-d `tensor_tensor_scan` (DVE) -> per-block local scans
              written into the y tile (used as scratch); keep only each
              block's last element (the block carry c).
      carry:  block-diagonal (per-h) triangular fp32 matmuls on TensorE
              propagate carries across blocks/tiles: C = sum TRI_ut^T @ c_u.
      pass 2: re-scan with initial = C[:, d], overwriting the y tile,
              then DMA out.
  - in-DMAs ride the Sync DGE, out-DMAs the Scalar DGE: separate FIFOs,
    so prefetches are not head-of-line blocked by output drains.
  - gamma-power matrices are precomputed on the host (gamma-derived
    constants only; all x-dependent work happens on device).
"""

import os

import numpy as np

import concourse.bacc as bacc
import concourse.bass as bass
import concourse.mybir as mybir
import concourse.tile as tile
from concourse.bass_utils import run_bass_kernel_spmd

F32 = mybir.dt.float32

B, H, S, D = 8, 16, 4096, 64
N_CORES = 8

# Per-core tiling: T s-tiles, BLK sequence blocks per tile, W steps per block.
T_TILES = 4
BLK = 8
W = S // (T_TILES * BLK)  # 128
NPART = H * BLK  # 128


def _pair_index(u, t):
    # index of (u, t), u <= t, in the stacked TRI tensor
    return t * (t + 1) // 2 + u


def build_program(T=T_TILES, blk=BLK, w=None, h=H, d=D, gp_split=0, ws=None):
    """Build the SPMD Bass program (same on every core).

    gp_split: number of d-chains (per pass, per tile) offloaded to GPSIMD.
    ws: optional per-tile block widths (list of T ints, sum*blk == S).
    """
    if ws is None:
        if w is None:
            w = S // (T * blk)
        ws = [w] * T
    T = len(ws)
    s = blk * sum(ws)
    npart = h * blk
    wmax = max(ws)
    npairs = T * (T + 1) // 2
    # per-tile start offsets in elements of the (h, s, d) tensor's s axis
    s_off = np.cumsum([0] + [blk * wi for wi in ws]).tolist()

    nc = bacc.Bacc("TRN2", target_bir_lowering=False, enable_partition_id=False)

    x_ext = nc.declare_dram_parameter("x", [h, s, d], F32, isOutput=False)
    gam_ext = nc.declare_dram_parameter(
        "gamma_tile", [npart, wmax], F32, isOutput=False
    )
    tri_ext = nc.declare_dram_parameter(
        "tri", [npairs, npart, npart], F32, isOutput=False
    )
    y_ext = nc.declare_dram_parameter("y", [h, s, d], F32, isOutput=True)

    # DRAM views per tile: (h, blk, w_t*d); iteration order (h, blk, wd)
    # matches the SBUF tile's (partition p = h*blk + blk, free = w*d) order.
    xf = x_ext[:].rearrange("h s d -> h (s d)")
    yf = y_ext[:].rearrange("h s d -> h (s d)")

    def tile_view(flat, t):
        wt = ws[t]
        v = flat[:, s_off[t] * d : s_off[t + 1] * d]
        return v.rearrange("h (blk wd) -> h blk wd", blk=blk)

    tri_v = tri_ext[:].rearrange("n k m -> k n m")

    mult = mybir.AluOpType.mult
    add = mybir.AluOpType.add

    with tile.TileContext(nc) as tc:
        with (
            tc.tile_pool(name="xp", bufs=2) as xp,
            tc.tile_pool(name="scratch", bufs=1) as sp,
            tc.tile_pool(name="consts", bufs=1) as cp,
            tc.tile_pool(name="psum", bufs=2, space="PSUM") as pp,
        ):
            gam = cp.tile([npart, wmax], F32)
            nc.sync.dma_start(gam[:], gam_ext[:])
            tri_sb = cp.tile([npart, npairs * npart], F32)
            nc.sync.dma_start(tri_sb[:], tri_v)
            # all block-carry vectors, one (npart, d) column-block per tile
            c_all = cp.tile([npart, T * d], F32)
            # SBUF copy of the propagated carries (scan initial source)
            cprop = cp.tile([npart, T * d], F32)

            scratch = sp.tile([npart, wmax * d], F32)

            for t in range(T):
                w = ws[t]
                free = w * d
                xt = xp.tile([npart, wmax * d], F32, tag="xt")
                nc.sync.dma_start(xt[:, :free], tile_view(xf, t))
                xt3 = xt[:, :free].rearrange("p (ww dd) -> p ww dd", dd=d)

                # pass-1 scratch: (p, d, w) view, each scan writes a
                # contiguous w-run; block carries are the (p, d, -1) slice
                scr3 = scratch[:, :free].rearrange("p (dd ww) -> p dd ww", dd=d)

                # pass 1: local scans (initial=0), keep only block carries
                for dd in range(d):
                    nc.vector.tensor_tensor_scan(
                        out=scr3[:, dd, :],
                        data0=gam[:, :w],
                        data1=xt3[:, :, dd],
                        initial=0.0,
                        op0=mult,
                        op1=add,
                    )
                nc.vector.tensor_copy(
                    c_all[:, t * d : (t + 1) * d], scr3[:, :, w - 1]
                )

                # carry propagation across blocks (and earlier tiles)
                C_t = pp.tile([npart, d], F32)
                for u in range(t + 1):
                    i = _pair_index(u, t)
                    nc.tensor.matmul(
                        C_t[:],
                        tri_sb[:, i * npart : (i + 1) * npart],
                        c_all[:, u * d : (u + 1) * d],
                        start=(u == 0),
                        stop=(u == t),
                    )
                # PSUM -> SBUF; the scan ISA op reads `initial` from here
                nc.vector.tensor_copy(cprop[:, t * d : (t + 1) * d], C_t[:])

                # pass 2: true scan with per-block initial carries, written
                # in place over the x tile (per-element read-then-write)
                for dd in range(d):
                    nc.vector.tensor_tensor_scan(
                        out=xt3[:, :, dd],
                        data0=gam[:, :w],
                        data1=xt3[:, :, dd],
                        initial=cprop[:, t * d + dd : t * d + dd + 1],
                        op0=mult,
                        op1=add,
                    )

                # out-DMA on the Scalar DGE (independent FIFO from inputs)
                nc.scalar.dma_start(tile_view(yf, t), xt[:, :free])

    # Run Bacc's lowering pipeline (incl. generate_event_semaphores, which
    # splits multi-sem waits to satisfy the one-wait-per-instruction
    # hardware constraint); the axon/pjrt exec path does not finalize
    # prebuilt modules itself.
    nc.finalize()
    return nc


def host_aux(gamma, T=T_TILES, blk=BLK, w=None, ws=None):
    """gamma-derived constant inputs (host precompute)."""
    if ws is None:
        if w is None:
            w = S // (T * blk)
        ws = [w] * T
    T = len(ws)
    h = gamma.shape[0]
    npart = h * blk
    wmax = max(ws)
    g64 = gamma.astype(np.float64)

    gamma_tile = np.repeat(gamma.astype(np.float32), blk)[:, None] * np.ones(
        (1, wmax), np.float32
    )

    # global block start offsets along s: block (t, p) spans
    # [start(t) + p*ws[t], start(t) + (p+1)*ws[t])
    tile_start = np.cumsum([0] + [blk * wi for wi in ws])

    def blk_start(t, p):
        return tile_start[t] + p * ws[t]

    def blk_end(t, p):  # inclusive last index
        return blk_start(t, p) + ws[t] - 1

    npairs = T * (T + 1) // 2
    tri = np.zeros((npairs, npart, npart), np.float64)
    # carry into block (t,p) from block (u,q): decay over the distance
    # from (u,q)'s last element to (t,p)'s first element minus one step
    with np.errstate(under="ignore"):
        for t in range(T):
            for u in range(t + 1):
                m = tri[_pair_index(u, t)]
                for q in range(blk):
                    for p in range(blk):
                        dist = blk_start(t, p) - 1 - blk_end(u, q)
                        if dist >= 0:
                            vals = g64**dist
                            for hh in range(h):
                                m[hh * blk + q, hh * blk + p] = vals[hh]
    return gamma_tile.astype(np.float32), tri.astype(np.float32)


_CACHE = {}

# production tiling: smaller first/last tiles shrink pipeline fill/drain
WS = [144, 224, 144]


def kernel(tensor, gamma):
    tensor = np.asarray(tensor, dtype=np.float32)
    gamma = np.asarray(gamma, dtype=np.float32)
    assert tensor.shape == (B, H, S, D), tensor.shape

    if "nc" not in _CACHE:
        _CACHE["nc"] = build_program(ws=WS)
    nc = _CACHE["nc"]

    gamma_tile, tri = host_aux(gamma, ws=WS)
    in_maps = [
        {"x": np.ascontiguousarray(tensor[b]), "gamma_tile": gamma_tile, "tri": tri}
        for b in range(N_CORES)
    ]
    res = run_bass_kernel_spmd(nc, in_maps, list(range(N_CORES)))
    out = np.stack([np.asarray(res.results[b]["y"]) for b in range(N_CORES)], axis=0)
    return out


# revision 42
# speedup vs baseline: 1.2791x; 1.0514x over previous
"""Discounted cumsum along S for tensor (8, 16, 4096, 64), gamma (16,).

y[b,h,t,d] = gamma[h] * y[b,h,t-1,d] + x[b,h,t,d],  y[...,-1,:] = 0

Strategy (8 NeuronCores, shard over B):
  - core b handles batch b: slab (16, 4096, 64) f32, 16 MiB in / 16 MiB out.
  - Per core: S=4096 split into T tiles; within a tile, partitions are
    (h, blk) = 16 heads x 8 sequence-blocks, each partition holding W
    consecutive s-steps x 64 d contiguous in DRAM (fast DMA).
  - Two-pass hierarchical scan:
      pass 1: per-d `tensor_tensor_scan` (DVE) -> per-block local scans
              written into the y tile (used as scratch); keep only each
              block's last element (the block carry c).
      carry:  block-diagonal (per-h) triangular fp32 matmuls on TensorE
              propagate carries across blocks/tiles: C = sum TRI_ut^T @ c_u.
      pass 2: re-scan with initial = C[:, d], overwriting the y tile,
              then DMA out.
  - in-DMAs ride the Sync DGE, out-DMAs the Scalar DGE: separate FIFOs,
    so prefetches are not head-of-line blocked by output drains.
  - gamma-power matrices are precomputed on the host (gamma-derived
    constants only; all x-dependent work happens on device).
"""

import os

import numpy as np

import concourse.bacc as bacc
import concourse.bass as bass
import concourse.mybir as mybir
import concourse.tile as tile
from concourse.bass_utils import run_bass_kernel_spmd

F32 = mybir.dt.float32

B, H, S, D = 8, 16, 4096, 64
N_CORES = 8

# Per-core tiling: T s-tiles, BLK sequence blocks per tile, W steps per block.
T_TILES = 4
BLK = 8
W = S // (T_TILES * BLK)  # 128
NPART = H * BLK  # 128


def _pair_index(u, t):
    # index of (u, t), u <= t, in the stacked TRI tensor
    return t * (t + 1) // 2 + u


def build_program(T=T_TILES, blk=BLK, w=None, h=H, d=D, gp_split=0, ws=None):
    """Build the SPMD Bass program (same on every core).

    gp_split: number of d-chains (per pass, per tile) offloaded to GPSIMD.
    ws: optional per-tile block widths (list of T ints, sum*blk == S).
    """
    if ws is None:
        if w is None:
            w = S // (T * blk)
        ws = [w] * T
    T = len(ws)
    s = blk * sum(ws)
    npart = h * blk
    wmax = max(ws)
    npairs = T * (T + 1) // 2
    # per-tile start offsets in elements of the (h, s, d) tensor's s axis
    s_off = np.cumsum([0] + [blk * wi for wi in ws]).tolist()

    nc = bacc.Bacc("TRN2", target_bir_lowering=False, enable_partition_id=False)

    x_ext = nc.declare_dram_parameter("x", [h, s, d], F32, isOutput=False)
    gam_ext = nc.declare_dram_parameter(
        "gamma_tile", [npart, wmax], F32, isOutput=False
    )
    tri_ext = nc.declare_dram_parameter(
        "tri", [npairs, npart, npart], F32, isOutput=False
    )
    y_ext = nc.declare_dram_parameter("y", [h, s, d], F32, isOutput=True)

    # DRAM views per tile: (h, blk, w_t*d); iteration order (h, blk, wd)
    # matches the SBUF tile's (partition p = h*blk + blk, free = w*d) order.
    xf = x_ext[:].rearrange("h s d -> h (s d)")
    yf = y_ext[:].rearrange("h s d -> h (s d)")

    def tile_view(flat, t):
        wt = ws[t]
        v = flat[:, s_off[t] * d : s_off[t + 1] * d]
        return v.rearrange("h (blk wd) -> h blk wd", blk=blk)

    tri_v = tri_ext[:].rearrange("n k m -> k n m")

    mult = mybir.AluOpType.mult
    add = mybir.AluOpType.add

    with tile.TileContext(nc) as tc:
        with (
            tc.tile_pool(name="xp", bufs=2) as xp,
            tc.tile_pool(name="scratch", bufs=1) as sp,
            tc.tile_pool(name="consts", bufs=1) as cp,
            tc.tile_pool(name="psum", bufs=2, space="PSUM") as pp,
        ):
            gam = cp.tile([npart, wmax], F32)
            nc.sync.dma_start(gam[:], gam_ext[:])
            # all block-carry vectors, one (npart, d) column-block per tile
            c_all = cp.tile([npart, T * d], F32)
            # SBUF copy of the propagated carries (scan initial source)
            cprop = cp.tile([npart, T * d], F32)

            scratch = sp.tile([npart, wmax * d], F32)

            tri_sb = cp.tile([npart, npairs * npart], F32)
            xts = [
                xp.tile([npart, wmax * d], F32, tag="xt", name=f"xt{i}")
                for i in range(T)
            ]
            for t in range(T):
                w = ws[t]
                free = w * d
                xt = xts[t]
                nc.sync.dma_start(xt[:, :free], tile_view(xf, t))
                if t == 0:
                    # tri is first needed by the t=0 carry matmul; issue its
                    # load after in-DMA(0) so the fill gets full bandwidth
                    nc.sync.dma_start(tri_sb[:], tri_v)
                xt3 = xt[:, :free].rearrange("p (ww dd) -> p ww dd", dd=d)

                # pass-1 scratch: (p, d, w) view, each scan writes a
                # contiguous w-run; block carries are the (p, d, -1) slice
                scr3 = scratch[:, :free].rearrange("p (dd ww) -> p dd ww", dd=d)

                # pass 1: local scans (initial=0), keep only block carries
                for dd in range(d):
                    nc.vector.tensor_tensor_scan(
                        out=scr3[:, dd, :],
                        data0=gam[:, :w],
                        data1=xt3[:, :, dd],
                        initial=0.0,
                        op0=mult,
                        op1=add,
                    )
                    if t == 0 and dd == 0 and T > 1:
                        # "touch" the next tile's buffer with a value that
                        # depends on the first scan: in-DMA(1) then waits on
                        # it (WAW), so in-DMA(0) fills at full bandwidth
                        nc.vector.tensor_copy(
                            xts[1][:, 0:1], scratch[:, 0:1]
                        )
                nc.vector.tensor_copy(
                    c_all[:, t * d : (t + 1) * d], scr3[:, :, w - 1]
                )

                # carry propagation across blocks (and earlier tiles)
                C_t = pp.tile([npart, d], F32)
                for u in range(t + 1):
                    i = _pair_index(u, t)
                    nc.tensor.matmul(
                        C_t[:],
                        tri_sb[:, i * npart : (i + 1) * npart],
                        c_all[:, u * d : (u + 1) * d],
                        start=(u == 0),
                        stop=(u == t),
                    )
                # PSUM -> SBUF; the scan ISA op reads `initial` from here
                nc.vector.tensor_copy(cprop[:, t * d : (t + 1) * d], C_t[:])

                # pass 2: true scan with per-block initial carries, written
                # in place over the x tile (per-element read-then-write)
                for dd in range(d):
                    nc.vector.tensor_tensor_scan(
                        out=xt3[:, :, dd],
                        data0=gam[:, :w],
                        data1=xt3[:, :, dd],
                        initial=cprop[:, t * d + dd : t * d + dd + 1],
                        op0=mult,
                        op1=add,
                    )

                # out-DMA on the Scalar DGE (independent FIFO from inputs)
                nc.scalar.dma_start(tile_view(yf, t), xt[:, :free])

    # Run Bacc's lowering pipeline (incl. generate_event_semaphores, which
    # splits multi-sem waits to satisfy the one-wait-per-instruction
    # hardware constraint); the axon/pjrt exec path does not finalize
    # prebuilt modules itself.
    nc.finalize()
    return nc


def host_aux(gamma, T=T_TILES, blk=BLK, w=None, ws=None):
    """gamma-derived constant inputs (host precompute)."""
    if ws is None:
        if w is None:
            w = S // (T * blk)
        ws = [w] * T
    T = len(ws)
    h = gamma.shape[0]
    npart = h * blk
    wmax = max(ws)
    g64 = gamma.astype(np.float64)

    gamma_tile = np.repeat(gamma.astype(np.float32), blk)[:, None] * np.ones(
        (1, wmax), np.float32
    )

    # global block start offsets along s: block (t, p) spans
    # [start(t) + p*ws[t], start(t) + (p+1)*ws[t])
    tile_start = np.cumsum([0] + [blk * wi for wi in ws])

    def blk_start(t, p):
        return tile_start[t] + p * ws[t]

    def blk_end(t, p):  # inclusive last index
        return blk_start(t, p) + ws[t] - 1

    npairs = T * (T + 1) // 2
    tri = np.zeros((npairs, npart, npart), np.float64)
    # carry into block (t,p) from block (u,q): decay over the distance
    # from (u,q)'s last element to (t,p)'s first element minus one step
    with np.errstate(under="ignore"):
        for t in range(T):
            for u in range(t + 1):
                m = tri[_pair_index(u, t)]
                for q in range(blk):
                    for p in range(blk):
                        dist = blk_start(t, p) - 1 - blk_end(u, q)
                        if dist >= 0:
                            vals = g64**dist
                            for hh in range(h):
                                m[hh * blk + q, hh * blk + p] = vals[hh]
    return gamma_tile.astype(np.float32), tri.astype(np.float32)


_CACHE = {}

# production tiling: smaller first/last tiles shrink pipeline fill/drain
WS = [96, 224, 144, 48]


def kernel(tensor, gamma):
    tensor = np.asarray(tensor, dtype=np.float32)
    gamma = np.asarray(gamma, dtype=np.float32)
    assert tensor.shape == (B, H, S, D), tensor.shape

    if "nc" not in _CACHE:
        _CACHE["nc"] = build_program(ws=WS)
    nc = _CACHE["nc"]

    gamma_tile, tri = host_aux(gamma, ws=WS)
    in_maps = [
        {"x": np.ascontiguousarray(tensor[b]), "gamma_tile": gamma_tile, "tri": tri}
        for b in range(N_CORES)
    ]
    res = run_bass_kernel_spmd(nc, in_maps, list(range(N_CORES)))
    out = np.stack([np.asarray(res.results[b]["y"]) for b in range(N_CORES)], axis=0)
    return out


# revision 44
# speedup vs baseline: 1.2808x; 1.0013x over previous
"""Discounted cumsum along S for tensor (8, 16, 4096, 64), gamma (16,).

y[b,h,t,d] = gamma[h] * y[b,h,t-1,d] + x[b,h,t,d],  y[...,-1,:] = 0

Strategy (8 NeuronCores, shard over B):
  - core b handles batch b: slab (16, 4096, 64) f32, 16 MiB in / 16 MiB out.
  - Per core: S=4096 split into T tiles; within a tile, partitions are
    (h, blk) = 16 heads x 8 sequence-blocks, each partition holding W
    consecutive s-steps x 64 d contiguous in DRAM (fast DMA).
  - Two-pass hierarchical scan:
      pass 1: per-d `tensor_tensor_scan` (DVE) -> per-block local scans
              written into the y tile (used as scratch); keep only each
              block's last element (the block carry c).
      carry:  block-diagonal (per-h) triangular fp32 matmuls on TensorE
              propagate carries across blocks/tiles: C = sum TRI_ut^T @ c_u.
      pass 2: re-scan with initial = C[:, d], overwriting the y tile,
              then DMA out.
  - in-DMAs ride the Sync DGE, out-DMAs the Scalar DGE: separate FIFOs,
    so prefetches are not head-of-line blocked by output drains.
  - gamma-power matrices are precomputed on the host (gamma-derived
    constants only; all x-dependent work happens on device).
"""

import os

import numpy as np

import concourse.bacc as bacc
import concourse.bass as bass
import concourse.mybir as mybir
import concourse.tile as tile
from concourse.bass_utils import run_bass_kernel_spmd

F32 = mybir.dt.float32

B, H, S, D = 8, 16, 4096, 64
N_CORES = 8

# Per-core tiling: T s-tiles, BLK sequence blocks per tile, W steps per block.
T_TILES = 4
BLK = 8
W = S // (T_TILES * BLK)  # 128
NPART = H * BLK  # 128


def _pair_index(u, t):
    # index of (u, t), u <= t, in the stacked TRI tensor
    return t * (t + 1) // 2 + u


def build_program(T=T_TILES, blk=BLK, w=None, h=H, d=D, gp_split=0, ws=None):
    """Build the SPMD Bass program (same on every core).

    gp_split: number of d-chains (per pass, per tile) offloaded to GPSIMD.
    ws: optional per-tile block widths (list of T ints, sum*blk == S).
    """
    if ws is None:
        if w is None:
            w = S // (T * blk)
        ws = [w] * T
    T = len(ws)
    s = blk * sum(ws)
    npart = h * blk
    wmax = max(ws)
    npairs = T * (T + 1) // 2
    # per-tile start offsets in elements of the (h, s, d) tensor's s axis
    s_off = np.cumsum([0] + [blk * wi for wi in ws]).tolist()

    nc = bacc.Bacc("TRN2", target_bir_lowering=False, enable_partition_id=False)

    x_ext = nc.declare_dram_parameter("x", [h, s, d], F32, isOutput=False)
    gam_ext = nc.declare_dram_parameter(
        "gamma_tile", [npart, wmax], F32, isOutput=False
    )
    tri_ext = nc.declare_dram_parameter(
        "tri", [npairs, npart, npart], F32, isOutput=False
    )
    y_ext = nc.declare_dram_parameter("y", [h, s, d], F32, isOutput=True)

    # DRAM views per tile: (h, blk, w_t*d); iteration order (h, blk, wd)
    # matches the SBUF tile's (partition p = h*blk + blk, free = w*d) order.
    xf = x_ext[:].rearrange("h s d -> h (s d)")
    yf = y_ext[:].rearrange("h s d -> h (s d)")

    def tile_view(flat, t):
        wt = ws[t]
        v = flat[:, s_off[t] * d : s_off[t + 1] * d]
        return v.rearrange("h (blk wd) -> h blk wd", blk=blk)

    tri_v = tri_ext[:].rearrange("n k m -> k n m")

    mult = mybir.AluOpType.mult
    add = mybir.AluOpType.add

    with tile.TileContext(nc) as tc:
        with (
            tc.tile_pool(name="xp", bufs=2) as xp,
            tc.tile_pool(name="scratch", bufs=1) as sp,
            tc.tile_pool(name="consts", bufs=1) as cp,
            tc.tile_pool(name="psum", bufs=2, space="PSUM") as pp,
        ):
            gam = cp.tile([npart, wmax], F32)
            nc.sync.dma_start(gam[:], gam_ext[:])
            # all block-carry vectors, one (npart, d) column-block per tile
            c_all = cp.tile([npart, T * d], F32)
            # SBUF copy of the propagated carries (scan initial source)
            cprop = cp.tile([npart, T * d], F32)

            scratch = sp.tile([npart, wmax * d], F32)

            tri_sb = cp.tile([npart, npairs * npart], F32)
            xts = [
                xp.tile([npart, wmax * d], F32, tag="xt", name=f"xt{i}")
                for i in range(T)
            ]
            for t in range(T):
                w = ws[t]
                free = w * d
                xt = xts[t]
                nc.sync.dma_start(xt[:, :free], tile_view(xf, t))
                if t == 0:
                    # tri is first needed by the t=0 carry matmul; issue its
                    # load after in-DMA(0) so the fill gets full bandwidth
                    nc.sync.dma_start(tri_sb[:], tri_v)
                xt3 = xt[:, :free].rearrange("p (ww dd) -> p ww dd", dd=d)

                # pass-1 scratch: (p, d, w) view, each scan writes a
                # contiguous w-run; block carries are the (p, d, -1) slice
                scr3 = scratch[:, :free].rearrange("p (dd ww) -> p dd ww", dd=d)

                # pass 1: local scans (initial=0), keep only block carries
                for dd in range(d):
                    nc.vector.tensor_tensor_scan(
                        out=scr3[:, dd, :],
                        data0=gam[:, 0:1].broadcast_to((npart, w)),
                        data1=xt3[:, :, dd],
                        initial=0.0,
                        op0=mult,
                        op1=add,
                    )
                    if t == 0 and dd == 0 and T > 1:
                        # "touch" the next tile's buffer with a value that
                        # depends on the first scan: in-DMA(1) then waits on
                        # it (WAW), so in-DMA(0) fills at full bandwidth
                        nc.vector.tensor_copy(
                            xts[1][:, 0:1], scratch[:, 0:1]
                        )
                nc.scalar.copy(
                    c_all[:, t * d : (t + 1) * d], scr3[:, :, w - 1]
                )

                # carry propagation across blocks (and earlier tiles)
                C_t = pp.tile([npart, d], F32)
                for u in range(t + 1):
                    i = _pair_index(u, t)
                    nc.tensor.matmul(
                        C_t[:],
                        tri_sb[:, i * npart : (i + 1) * npart],
                        c_all[:, u * d : (u + 1) * d],
                        start=(u == 0),
                        stop=(u == t),
                    )
                # PSUM -> SBUF; the scan ISA op reads `initial` from here
                nc.scalar.copy(cprop[:, t * d : (t + 1) * d], C_t[:])

                # pass 2: true scan with per-block initial carries, written
                # in place over the x tile (per-element read-then-write)
                for dd in range(d):
                    nc.vector.tensor_tensor_scan(
                        out=xt3[:, :, dd],
                        data0=gam[:, 0:1].broadcast_to((npart, w)),
                        data1=xt3[:, :, dd],
                        initial=cprop[:, t * d + dd : t * d + dd + 1],
                        op0=mult,
                        op1=add,
                    )

                # out-DMA on the Scalar DGE (independent FIFO from inputs)
                nc.scalar.dma_start(tile_view(yf, t), xt[:, :free])

    # Run Bacc's lowering pipeline (incl. generate_event_semaphores, which
    # splits multi-sem waits to satisfy the one-wait-per-instruction
    # hardware constraint); the axon/pjrt exec path does not finalize
    # prebuilt modules itself.
    nc.finalize()
    return nc


def host_aux(gamma, T=T_TILES, blk=BLK, w=None, ws=None):
    """gamma-derived constant inputs (host precompute)."""
    if ws is None:
        if w is None:
            w = S // (T * blk)
        ws = [w] * T
    T = len(ws)
    h = gamma.shape[0]
    npart = h * blk
    wmax = max(ws)
    g64 = gamma.astype(np.float64)

    gamma_tile = np.repeat(gamma.astype(np.float32), blk)[:, None] * np.ones(
        (1, wmax), np.float32
    )

    # global block start offsets along s: block (t, p) spans
    # [start(t) + p*ws[t], start(t) + (p+1)*ws[t])
    tile_start = np.cumsum([0] + [blk * wi for wi in ws])

    def blk_start(t, p):
        return tile_start[t] + p * ws[t]

    def blk_end(t, p):  # inclusive last index
        return blk_start(t, p) + ws[t] - 1

    npairs = T * (T + 1) // 2
    tri = np.zeros((npairs, npart, npart), np.float64)
    # carry into block (t,p) from block (u,q): decay over the distance
    # from (u,q)'s last element to (t,p)'s first element minus one step
    with np.errstate(under="ignore"):
        for t in range(T):
            for u in range(t + 1):
                m = tri[_pair_index(u, t)]
                for q in range(blk):
                    for p in range(blk):
                        dist = blk_start(t, p) - 1 - blk_end(u, q)
                        if dist >= 0:
                            vals = g64**dist
                            for hh in range(h):
                                m[hh * blk + q, hh * blk + p] = vals[hh]
    return gamma_tile.astype(np.float32), tri.astype(np.float32)


_CACHE = {}

# production tiling: smaller first/last tiles shrink pipeline fill/drain
WS = [96, 224, 144, 48]


def kernel(tensor, gamma):
    tensor = np.asarray(tensor, dtype=np.float32)
    gamma = np.asarray(gamma, dtype=np.float32)
    assert tensor.shape == (B, H, S, D), tensor.shape

    if "nc" not in _CACHE:
        _CACHE["nc"] = build_program(ws=WS)
    nc = _CACHE["nc"]

    gamma_tile, tri = host_aux(gamma, ws=WS)
    in_maps = [
        {"x": np.ascontiguousarray(tensor[b]), "gamma_tile": gamma_tile, "tri": tri}
        for b in range(N_CORES)
    ]
    res = run_bass_kernel_spmd(nc, in_maps, list(range(N_CORES)))
    out = np.stack([np.asarray(res.results[b]["y"]) for b in range(N_CORES)], axis=0)
    return out
